# revision 1
# baseline (speedup 1.0000x reference)
"""AUGRU (attention-gated GRU) Trainium2 Bass kernel.

Math (per batch row b, per step t, E=10):
    r = sigmoid((x@Wi_r + bi_r + h@Wh_r) @ Ws_r + bs_r)
    z = sigmoid((x@Wi_z + bi_z + h@Wh_z) @ Ws_z + bs_z)
    hc = tanh((x@Wi_h + bi_h + (h*z)@Wh_h) @ Wt_h + bt_h)
    Ra = a * r
    h' = (1-Ra)*h + Ra*hc

The double-dense per gate folds algebraically:
    r = sigmoid(x@(Wi_r@Ws_r) + h@(Wh_r@Ws_r) + (bi_r@Ws_r + bs_r))
so only single effective 10x10 matrices reach the device.

Device layout (per core, batch shard BC=8192 padded to 8256):
  batch b -> block q = b // 688, column j = b % 688 (group g = j // 344)
  partition p = q*10 + e  (12 blocks x 10 feats = 120 partitions)
Matmuls are block-diagonal [120(+1 ones row), 120] stationaries on PE;
columns (batch) stream through. Host pre-transposes x/a to [T, 121, 688]
so all DMAs are contiguous. Two column groups (g=0,1) pipeline the serial
per-step dependency chain across engines.
"""

import os
from contextlib import ExitStack

import numpy as np

B, T, E = 65536, 50, 10
NCORES = 8
BC = B // NCORES          # 8192
NB = 12                   # batch blocks per core
P = NB * E                # 120 partitions
NCOL = 688                # columns per core (NB*NCOL = 8256 >= BC)
BPAD = NB * NCOL          # 8256
NG = 2                    # column pipeline groups
CG = NCOL // NG           # 344
TCH = 10                  # timesteps per X/A DMA chunk

_DT = os.environ.get("AUGRU_DT", "f16")   # f32 | f16 | bf16
_WBUFS = int(os.environ.get("AUGRU_WBUFS", "8"))

_cache = {}


def _np_dtype():
    if _DT == "f32":
        return np.float32
    if _DT == "f16":
        return np.float16
    import ml_dtypes
    return ml_dtypes.bfloat16


def _build_program(reps=1, loop=False, ablate=()):
    import concourse.bacc as bacc
    import concourse.tile as tile
    from concourse import mybir

    mdt = {
        "f32": mybir.dt.float32,
        "f16": mybir.dt.float16,
        "bf16": mybir.dt.bfloat16,
    }[_DT]
    f32 = mybir.dt.float32
    SIG = mybir.ActivationFunctionType.Sigmoid
    TANH = mybir.ActivationFunctionType.Tanh
    MULT = mybir.AluOpType.mult
    ADD = mybir.AluOpType.add
    SUB = mybir.AluOpType.subtract
    ab = set(ablate)

    nc = bacc.Bacc("TRN2", target_bir_lowering=False, debug=False)

    NCH = T // TCH
    xd = nc.dram_tensor("xd", [NCH, P + 1, TCH * NCOL], mdt, kind="ExternalInput")
    ad = nc.dram_tensor("ad", [NCH, P, TCH * NCOL], mdt, kind="ExternalInput")
    h0d = nc.dram_tensor("h0d", [P, NCOL], mdt, kind="ExternalInput")
    lwx = nc.dram_tensor("lwx", [P + 1, 3 * P], mdt, kind="ExternalInput")
    lwh = nc.dram_tensor("lwh", [P, 5 * P], mdt, kind="ExternalInput")
    outd = nc.dram_tensor("outd", [P, NCOL], mdt, kind="ExternalOutput")

    with tile.TileContext(nc) as tc, ExitStack() as ctx:
        cpool = ctx.enter_context(tc.tile_pool(name="consts", bufs=1))
        xpool = ctx.enter_context(tc.tile_pool(name="xbuf", bufs=2))
        apool = ctx.enter_context(tc.tile_pool(name="abuf", bufs=2))
        hpool = ctx.enter_context(tc.tile_pool(name="hbuf", bufs=1))
        wpool = ctx.enter_context(tc.tile_pool(name="work", bufs=_WBUFS))
        ppool = ctx.enter_context(tc.tile_pool(name="psum", bufs=2, space="PSUM"))

        # stationary weights, resident in SBUF
        lx = cpool.tile([P + 1, 3 * P], mdt, tag="lx", name="lx")
        lh = cpool.tile([P, 5 * P], mdt, tag="lh", name="lh")
        nc.sync.dma_start(out=lx[:, :], in_=lwx[:, :])
        nc.sync.dma_start(out=lh[:, :], in_=lwh[:, :])
        L_zx, L_rx, L_hx = (lx[:, i * P:(i + 1) * P] for i in range(3))
        L_zh, L_rh, L_ah, L_zhn, L_rhn = (lh[:, i * P:(i + 1) * P] for i in range(5))

        # recurrent state, parity double-buffered
        h_tiles = [
            hpool.tile([P, NCOL], mdt, tag=f"h{i}", name=f"h{i}") for i in range(2)
        ]
        nc.sync.dma_start(out=h_tiles[0][:, :], in_=h0d[:, :])
        if "chain" in ab:
            nc.sync.dma_start(out=h_tiles[1][:, :], in_=h0d[:, :])

        state = {"xt": None, "at": None, "m1p": None, "m2": None}

        def emit_body():
            for t in range(T):
                if t % TCH == 0:
                    ci = t // TCH
                    state["xt"] = xpool.tile(
                        [P + 1, TCH, NCOL], mdt, tag="xt", name="xt"
                    )
                    nc.sync.dma_start(
                        out=state["xt"][:, :, :],
                        in_=xd[ci, :, :].rearrange("p (t c) -> p t c", t=TCH),
                    )
                    state["at"] = apool.tile([P, TCH, NCOL], mdt, tag="at", name="at")
                    nc.sync.dma_start(
                        out=state["at"][:, :, :],
                        in_=ad[ci, :, :].rearrange("p (t c) -> p t c", t=TCH),
                    )
                xt, at = state["xt"], state["at"]
                tt = t % TCH
                cur, nxt = h_tiles[t % 2], h_tiles[(t + 1) % 2]

                X = [xt[0:P + 1, tt, g * CG:(g + 1) * CG] for g in range(NG)]
                A = [at[:, tt, g * CG:(g + 1) * CG] for g in range(NG)]
                H = [cur[:, g * CG:(g + 1) * CG] for g in range(NG)]
                HN = [nxt[:, g * CG:(g + 1) * CG] for g in range(NG)]

                pz = [
                    ppool.tile([P, 2, 512], f32, tag="pz", name="pz")
                    for _ in range(NG)
                ]
                px = [
                    ppool.tile([P, 512], f32, tag="px", name="px") for _ in range(NG)
                ]

                if "pe2" in ab:
                    pdup = ppool.tile([P, 2, 512], f32, tag="pdup", name="pdup", bufs=1)
                    for g in range(NG):
                        nc.tensor.matmul(pdup[:, 0, 0:CG], L_zx, X[g], start=True, stop=True)
                        nc.tensor.matmul(pdup[:, 1, 0:CG], L_rx, X[g], start=True, stop=True)
                        nc.tensor.matmul(pdup[:, 0, 0:CG], L_hx, X[g], start=True, stop=True)
                        nc.tensor.matmul(pdup[:, 1, 0:CG], L_zh, H[g], start=True, stop=True)
                        nc.tensor.matmul(pdup[:, 0, 0:CG], L_rh, H[g], start=True, stop=True)
                        nc.tensor.matmul(pdup[:, 1, 0:CG], L_ah, H[g], start=True, stop=True)
                if "dma2" in ab and t % TCH == 0:
                    nc.sync.dma_start(
                        out=state["xt"][:, :, :],
                        in_=xd[ci, :, :].rearrange("p (t c) -> p t c", t=TCH),
                    )
                    nc.sync.dma_start(
                        out=state["at"][:, :, :],
                        in_=ad[ci, :, :].rearrange("p (t c) -> p t c", t=TCH),
                    )
                # x-side matmuls: no dependency on h(t-1); PE can run these early
                if "pe" not in ab:
                    for g in range(NG):
                        nc.tensor.matmul(
                            pz[g][:, 0, 0:CG], L_zx, X[g], start=True, stop=False
                        )
                    for g in range(NG):
                        nc.tensor.matmul(
                            pz[g][:, 1, 0:CG], L_rx, X[g], start=True, stop=False
                        )
                    for g in range(NG):
                        nc.tensor.matmul(
                            px[g][:, 0:CG], L_hx, X[g], start=True, stop=False
                        )
                    # h-side accumulation. h(t-1) = m2 - m1p enters as two
                    # accumulating matmuls (m1p with negated weights), so the
                    # h materialization (DVE) is off the critical path.
                    if t == 0 and state["m2"] is None:
                        for g in range(NG):
                            nc.tensor.matmul(
                                pz[g][:, 0, 0:CG], L_zh, H[g], start=False, stop=True
                            )
                            nc.tensor.matmul(
                                pz[g][:, 1, 0:CG], L_rh, H[g], start=False, stop=True
                            )
                    else:
                        pm1, pm2 = state["m1p"], state["m2"]
                        for g in range(NG):
                            nc.tensor.matmul(
                                pz[g][:, 0, 0:CG], L_zhn, pm1[g][:, :],
                                start=False, stop=False,
                            )
                            nc.tensor.matmul(
                                pz[g][:, 1, 0:CG], L_rhn, pm1[g][:, :],
                                start=False, stop=False,
                            )
                        for g in range(NG):
                            nc.tensor.matmul(
                                pz[g][:, 0, 0:CG], L_zh, pm2[g][:, :],
                                start=False, stop=True,
                            )
                            nc.tensor.matmul(
                                pz[g][:, 1, 0:CG], L_rh, pm2[g][:, :],
                                start=False, stop=True,
                            )

                zr, hz, ra, hc, m1p, m2 = ({} for _ in range(6))
                for g in range(NG):
                    zr[g] = wpool.tile([P, 2, CG], mdt, tag="zr", name="zr")
                    if "act" not in ab:
                        nc.scalar.activation(zr[g][:, :, :], pz[g][:, :, 0:CG], SIG)
                    if "act2" in ab:
                        nc.scalar.activation(zr[g][:, :, :], pz[g][:, :, 0:CG], SIG)
                for g in range(NG):
                    hz[g] = wpool.tile([P, CG], mdt, tag="hz", name="hz")
                    hz_eng = nc.gpsimd if "hzgp" in ab else nc.vector
                    if "dve" not in ab:
                        hz_eng.tensor_tensor(hz[g][:, :], H[g], zr[g][:, 0, :], MULT)
                    if "dve2" in ab:
                        hz_eng.tensor_tensor(hz[g][:, :], H[g], zr[g][:, 0, :], MULT)
                if "pe" not in ab:
                    for g in range(NG):
                        nc.tensor.matmul(
                            px[g][:, 0:CG], L_ah, hz[g][:, :], start=False, stop=True
                        )
                for g in range(NG):
                    ra[g] = wpool.tile([P, CG], mdt, tag="ra", name="ra")
                    ra_eng = nc.gpsimd if "ragp" in ab else nc.vector
                    if "dve" not in ab:
                        ra_eng.tensor_tensor(ra[g][:, :], A[g], zr[g][:, 1, :], MULT)
                    if "dve2" in ab:
                        ra_eng.tensor_tensor(ra[g][:, :], A[g], zr[g][:, 1, :], MULT)
                for g in range(NG):
                    # m1p = (ra - 1) * h = -(1-ra)*h, fused on gpsimd
                    m1p[g] = wpool.tile([P, CG], mdt, tag="m1p", name="m1p")
                    if "dve" not in ab:
                        nc.vector.scalar_tensor_tensor(
                            m1p[g][:, :], ra[g][:, :], 1.0, H[g], SUB, MULT
                        )
                    if "dve2" in ab:
                        nc.vector.scalar_tensor_tensor(
                            m1p[g][:, :], ra[g][:, :], 1.0, H[g], SUB, MULT
                        )
                for g in range(NG):
                    hc[g] = wpool.tile([P, CG], mdt, tag="hc", name="hc")
                    if "act" not in ab:
                        nc.scalar.activation(hc[g][:, :], px[g][:, 0:CG], TANH)
                    if "act2" in ab:
                        nc.scalar.activation(hc[g][:, :], px[g][:, 0:CG], TANH)
                for g in range(NG):
                    m2[g] = wpool.tile([P, CG], mdt, tag="m2", name="m2")
                    m2_eng = nc.gpsimd if "m2gp" in ab else nc.vector
                    if "dve" not in ab:
                        m2_eng.tensor_tensor(m2[g][:, :], ra[g][:, :], hc[g][:, :], MULT)
                    if "dve2" in ab:
                        m2_eng.tensor_tensor(m2[g][:, :], ra[g][:, :], hc[g][:, :], MULT)
                # materialized h' = m2 - m1p, off the matmul critical path
                hp_eng = nc.vector if "hpdve" in ab else nc.gpsimd
                if "chain" not in ab and "gp" not in ab:
                    for g in range(NG):
                        hp_eng.tensor_tensor(HN[g], m2[g][:, :], m1p[g][:, :], SUB)
                if "gp2" in ab:
                    for g in range(NG):
                        nc.gpsimd.tensor_tensor(HN[g], m2[g][:, :], m1p[g][:, :], SUB)
                state["m1p"], state["m2"] = m1p, m2

        if loop and reps > 1:
            with tc.For_i(0, reps, 1):
                emit_body()
        else:
            for _ in range(reps):
                emit_body()

        nc.sync.dma_start(out=outd[:, :], in_=h_tiles[T % 2][:, :])

    nc.compile()
    return nc


def _get_program(reps=1, loop=False, ablate=()):
    key = ("nc", reps, loop, tuple(sorted(ablate)))
    if key not in _cache:
        _cache[key] = _build_program(reps, loop, ablate)
    return _cache[key]


def _effective_weights(inp):
    """Fold the per-gate double-dense into single 10x10 matrices (float64)."""
    g = {k: np.asarray(v, np.float64) for k, v in inp.items()}
    Wxz = g["Wi_z"] @ g["Ws_z"]; Az = g["Wh_z"] @ g["Ws_z"]
    bz = g["bi_z"] @ g["Ws_z"] + g["bs_z"]
    Wxr = g["Wi_r"] @ g["Ws_r"]; Ar = g["Wh_r"] @ g["Ws_r"]
    br = g["bi_r"] @ g["Ws_r"] + g["bs_r"]
    Wxh = g["Wi_h"] @ g["Wt_h"]; Ah = g["Wh_h"] @ g["Wt_h"]
    bh = g["bi_h"] @ g["Wt_h"] + g["bt_h"]
    return (Wxz, Az, bz), (Wxr, Ar, br), (Wxh, Ah, bh)


def _block_diag_lhsT(W, bias=None):
    """[P(+1), P] stationary: block-diagonal W per batch block, bias on ones row."""
    rows = P + 1 if bias is not None else P
    L = np.zeros((rows, P), np.float64)
    for q in range(NB):
        L[q * E:(q + 1) * E, q * E:(q + 1) * E] = W
        if bias is not None:
            L[P, q * E:(q + 1) * E] = bias
    return L


def _to_device_layout(arr, dt, ones_row=False):
    """[BPAD, T, E] -> [T//TCH, P(+1), TCH*NCOL]: chunk-major, contiguous
    per (chunk, partition) so each chunk DMA is one descriptor/partition."""
    out = arr.reshape(NB, NCOL, T, E).transpose(2, 0, 3, 1).reshape(T, P, NCOL)
    if ones_row:
        out = np.concatenate([out, np.ones((T, 1, NCOL), out.dtype)], axis=1)
    p = out.shape[1]
    # [T, p, NCOL] -> [NCH, TCH, p, NCOL] -> [NCH, p, TCH, NCOL] -> flat
    out = out.reshape(T // TCH, TCH, p, NCOL).transpose(0, 2, 1, 3).reshape(
        T // TCH, p, TCH * NCOL
    )
    return np.ascontiguousarray(out).astype(dt)


def _prep_in_maps(inputs):
    dt = _np_dtype()
    x = np.asarray(inputs["gru_hidden_state_inputs"], np.float32)
    a = np.asarray(inputs["attention_s"], np.float32)
    h0 = np.asarray(inputs["h0"], np.float32)

    (Wxz, Az, bz), (Wxr, Ar, br), (Wxh, Ah, bh) = _effective_weights(inputs)
    LWX = np.concatenate(
        [_block_diag_lhsT(Wxz, bz), _block_diag_lhsT(Wxr, br), _block_diag_lhsT(Wxh, bh)],
        axis=1,
    ).astype(dt)                                   # [121, 360]
    Lzh, Lrh, Lah = _block_diag_lhsT(Az), _block_diag_lhsT(Ar), _block_diag_lhsT(Ah)
    LWH = np.concatenate([Lzh, Lrh, Lah, -Lzh, -Lrh], axis=1).astype(dt)  # [120, 600]
    H0 = np.tile(h0.reshape(1, E, 1).astype(np.float64), (NB, 1, NCOL)).reshape(
        P, NCOL
    ).astype(dt)

    pad = BPAD - BC
    in_maps = []
    for c in range(NCORES):
        xc = np.concatenate([x[c * BC:(c + 1) * BC], np.zeros((pad, T, E), np.float32)])
        ac = np.concatenate([a[c * BC:(c + 1) * BC], np.zeros((pad, T, E), np.float32)])
        in_maps.append({
            "xd": _to_device_layout(xc, dt, ones_row=True),
            "ad": _to_device_layout(ac, dt),
            "h0d": H0,
            "lwx": LWX,
            "lwh": LWH,
        })
    return in_maps


def kernel(**inputs):
    in_maps = _prep_in_maps(inputs)
    nc = _get_program()
    put, run, _ = _get_runner(nc, 1)
    outs = run(put(in_maps))

    out = np.empty((B, E), np.float32)
    for c in range(NCORES):
        o = np.asarray(outs[c], np.float32)   # [P, NCOL]
        o = o.reshape(NB, E, NCOL).transpose(0, 2, 1).reshape(BPAD, E)
        out[c * BC:(c + 1) * BC] = o[:BC]
    return out


def _make_runner(nc):
    """Reusable jitted executor for `nc` (mirrors bass2jax.run_bass_via_pjrt
    multi-core path, but keeps the jitted fn for re-timing)."""
    import jax
    from jax.sharding import Mesh, PartitionSpec
    from jax.experimental.shard_map import shard_map
    from concourse import bass2jax, mybir

    bass2jax.install_neuronx_cc_hook()
    n_cores = NCORES

    in_names, out_names, out_avals = [], [], []
    for alloc in nc.m.functions[0].allocations:
        if not isinstance(alloc, mybir.MemoryLocationSet):
            continue
        name = alloc.memorylocations[0].name
        if alloc.kind == "ExternalInput":
            if nc.partition_id_tensor is not None and name == nc.partition_id_tensor.name:
                continue
            in_names.append(name)
        elif alloc.kind == "ExternalOutput":
            out_names.append(name)
            out_avals.append(
                jax.core.ShapedArray(tuple(alloc.tensor_shape), mybir.dt.np(alloc.dtype))
            )
    n_params = len(in_names)
    all_in_names = list(in_names) + list(out_names)
    if nc.partition_id_tensor is not None:
        all_in_names.append(nc.partition_id_tensor.name)

    def _body(*args):
        operands = list(args)
        if nc.partition_id_tensor is not None:
            operands.append(bass2jax.partition_id_tensor())
        outs = bass2jax._bass_exec_p.bind(
            *operands,
            out_avals=tuple(out_avals),
            in_names=tuple(all_in_names),
            out_names=tuple(out_names),
            lowering_input_output_aliases=(),
            sim_require_finite=True,
            sim_require_nnan=True,
            nc=nc,
        )
        return tuple(outs)

    devices = jax.devices()[:n_cores]
    mesh = Mesh(np.asarray(devices), ("core",))
    in_specs = (PartitionSpec("core"),) * (n_params + len(out_names))
    out_specs = (PartitionSpec("core"),) * len(out_names)
    sharded = jax.jit(
        shard_map(_body, mesh=mesh, in_specs=in_specs, out_specs=out_specs,
                  check_rep=False),
        keep_unused=True,
    )

    def put(in_maps):
        concat_in = [
            np.concatenate([np.asarray(in_maps[c][name]) for c in range(n_cores)], axis=0)
            for name in in_names
        ]
        concat_zeros = [
            np.zeros((n_cores * a.shape[0], *a.shape[1:]), a.dtype) for a in out_avals
        ]
        return [jax.device_put(a) for a in (*concat_in, *concat_zeros)]

    def run_device(dev_args):
        outs = sharded(*dev_args)
        return jax.block_until_ready(outs)

    def fetch(outs):
        o = np.asarray(outs[0]).reshape(n_cores, *out_avals[0].shape)
        return [o[c] for c in range(n_cores)]

    def run(dev_args):
        return fetch(run_device(dev_args))

    return put, run, run_device


def _get_runner(nc, key):
    k = ("runner", key)
    if k not in _cache:
        _cache[k] = _make_runner(nc)
    return _cache[k]



# revision 8
# speedup vs baseline: 1.1235x; 1.1235x over previous
"""AUGRU (attention-gated GRU) Trainium2 Bass kernel.

Math (per batch row b, per step t, E=10):
    r = sigmoid((x@Wi_r + bi_r + h@Wh_r) @ Ws_r + bs_r)
    z = sigmoid((x@Wi_z + bi_z + h@Wh_z) @ Ws_z + bs_z)
    hc = tanh((x@Wi_h + bi_h + (h*z)@Wh_h) @ Wt_h + bt_h)
    Ra = a * r
    h' = (1-Ra)*h + Ra*hc

The double-dense per gate folds algebraically:
    r = sigmoid(x@(Wi_r@Ws_r) + h@(Wh_r@Ws_r) + (bi_r@Ws_r + bs_r))
so only single effective 10x10 matrices reach the device.

Device layout (per core, batch shard BC=8192 padded to 8256):
  batch b -> block q = b // 688, column j = b % 688 (group g = j // 344)
  partition p = q*10 + e  (12 blocks x 10 feats = 120 partitions)
Matmuls are block-diagonal [120(+1 ones row), 120] stationaries on PE;
columns (batch) stream through. Host pre-transposes x/a to [T, 121, 688]
so all DMAs are contiguous. Two column groups (g=0,1) pipeline the serial
per-step dependency chain across engines.
"""

import os
from contextlib import ExitStack

import numpy as np

B, T, E = 65536, 50, 10
NCORES = 8
BC = B // NCORES          # 8192
NB = 12                   # batch blocks per core
P = NB * E                # 120 partitions
NCOL = 688                # columns per core (NB*NCOL = 8256 >= BC)
BPAD = NB * NCOL          # 8256
NG = 2                    # column pipeline groups
CG = NCOL // NG           # 344
TCH = 10                  # timesteps per X/A DMA chunk

_DT = os.environ.get("AUGRU_DT", "f16")   # f32 | f16 | bf16
_WBUFS = int(os.environ.get("AUGRU_WBUFS", "8"))

_cache = {}


def _np_dtype():
    if _DT == "f32":
        return np.float32
    if _DT == "f16":
        return np.float16
    import ml_dtypes
    return ml_dtypes.bfloat16


def _build_program(reps=1, loop=False, ablate=()):
    import concourse.bacc as bacc
    import concourse.tile as tile
    from concourse import mybir

    mdt = {
        "f32": mybir.dt.float32,
        "f16": mybir.dt.float16,
        "bf16": mybir.dt.bfloat16,
    }[_DT]
    f32 = mybir.dt.float32
    SIG = mybir.ActivationFunctionType.Sigmoid
    TANH = mybir.ActivationFunctionType.Tanh
    MULT = mybir.AluOpType.mult
    ADD = mybir.AluOpType.add
    SUB = mybir.AluOpType.subtract
    ab = set(ablate)

    nc = bacc.Bacc("TRN2", target_bir_lowering=False, debug=False)

    NCH = T // TCH
    xd = nc.dram_tensor("xd", [NCH, P + 1, TCH * NCOL], mdt, kind="ExternalInput")
    ad = nc.dram_tensor("ad", [NCH, P, TCH * NCOL], mdt, kind="ExternalInput")
    h0d = nc.dram_tensor("h0d", [P, NCOL], mdt, kind="ExternalInput")
    lwx = nc.dram_tensor("lwx", [P + 1, 3 * P], mdt, kind="ExternalInput")
    lwh = nc.dram_tensor("lwh", [P, 5 * P], mdt, kind="ExternalInput")
    outd = nc.dram_tensor("outd", [P, NCOL], mdt, kind="ExternalOutput")

    with tile.TileContext(nc) as tc, ExitStack() as ctx:
        cpool = ctx.enter_context(tc.tile_pool(name="consts", bufs=1))
        xpool = ctx.enter_context(tc.tile_pool(name="xbuf", bufs=2))
        apool = ctx.enter_context(tc.tile_pool(name="abuf", bufs=2))
        hpool = ctx.enter_context(tc.tile_pool(name="hbuf", bufs=1))
        wpool = ctx.enter_context(tc.tile_pool(name="work", bufs=_WBUFS))
        ppool = ctx.enter_context(tc.tile_pool(name="psum", bufs=2, space="PSUM"))

        # stationary weights, resident in SBUF
        lx = cpool.tile([P + 1, 3 * P], mdt, tag="lx", name="lx")
        lh = cpool.tile([P, 5 * P], mdt, tag="lh", name="lh")
        nc.sync.dma_start(out=lx[:, :], in_=lwx[:, :])
        nc.sync.dma_start(out=lh[:, :], in_=lwh[:, :])
        L_zx, L_rx, L_hx = (lx[:, i * P:(i + 1) * P] for i in range(3))
        L_zh, L_rh, L_ah, L_zhn, L_rhn = (lh[:, i * P:(i + 1) * P] for i in range(5))

        # recurrent state, parity double-buffered
        h_tiles = [
            hpool.tile([P, NCOL], mdt, tag=f"h{i}", name=f"h{i}") for i in range(2)
        ]
        nc.sync.dma_start(out=h_tiles[0][:, :], in_=h0d[:, :])
        if "chain" in ab:
            nc.sync.dma_start(out=h_tiles[1][:, :], in_=h0d[:, :])

        state = {"xt": None, "at": None, "m1p": None, "m2": None}
        mat_h = "matH" in ab         # z/r h-side from materialized h (2 MMs fewer)
        act_ms = "actmergeS" in ab   # one sigmoid instr across both groups
        act_mt = "actmergeT" in ab   # one tanh instr across both groups

        def emit_body():
            for t in range(T):
                if t % TCH == 0:
                    ci = t // TCH
                    state["xt"] = xpool.tile(
                        [P + 1, TCH, NCOL], mdt, tag="xt", name="xt"
                    )
                    nc.sync.dma_start(
                        out=state["xt"][:, :, :],
                        in_=xd[ci, :, :].rearrange("p (t c) -> p t c", t=TCH),
                    )
                    state["at"] = apool.tile([P, TCH, NCOL], mdt, tag="at", name="at")
                    nc.sync.dma_start(
                        out=state["at"][:, :, :],
                        in_=ad[ci, :, :].rearrange("p (t c) -> p t c", t=TCH),
                    )
                xt, at = state["xt"], state["at"]
                tt = t % TCH
                cur, nxt = h_tiles[t % 2], h_tiles[(t + 1) % 2]

                X = [xt[0:P + 1, tt, g * CG:(g + 1) * CG] for g in range(NG)]
                A = [at[:, tt, g * CG:(g + 1) * CG] for g in range(NG)]
                H = [cur[:, g * CG:(g + 1) * CG] for g in range(NG)]
                HN = [nxt[:, g * CG:(g + 1) * CG] for g in range(NG)]

                if act_ms:
                    pzm = ppool.tile([P, 2, 2, 512], f32, tag="pz", name="pz", bufs=1)
                    zps = lambda g, j, c=CG: pzm[:, g, j, 0:c]
                else:
                    pzb = 3 if "pz3" in ab else 2
                    pz = [
                        ppool.tile([P, 2, 512], f32, tag="pz", name="pz", bufs=pzb)
                        for _ in range(NG)
                    ]
                    zps = lambda g, j, c=CG: pz[g][:, j, 0:c]
                if act_mt:
                    pxb = 1 if act_ms else 2
                    pxm = ppool.tile([P, 2, 512], f32, tag="px", name="px", bufs=pxb)
                    cps = lambda g, c=CG: pxm[:, g, 0:c]
                else:
                    px = [
                        ppool.tile([P, 512], f32, tag="px", name="px")
                        for _ in range(NG)
                    ]
                    cps = lambda g, c=CG: px[g][:, 0:c]

                if "pe2" in ab:
                    pdup = ppool.tile([P, 2, 512], f32, tag="pdup", name="pdup", bufs=1)
                    for g in range(NG):
                        nc.tensor.matmul(pdup[:, 0, 0:CG], L_zx, X[g], start=True, stop=True)
                        nc.tensor.matmul(pdup[:, 1, 0:CG], L_rx, X[g], start=True, stop=True)
                        nc.tensor.matmul(pdup[:, 0, 0:CG], L_hx, X[g], start=True, stop=True)
                        nc.tensor.matmul(pdup[:, 1, 0:CG], L_zh, H[g], start=True, stop=True)
                        nc.tensor.matmul(pdup[:, 0, 0:CG], L_rh, H[g], start=True, stop=True)
                        nc.tensor.matmul(pdup[:, 1, 0:CG], L_ah, H[g], start=True, stop=True)
                if "dma2" in ab and t % TCH == 0:
                    nc.sync.dma_start(
                        out=state["xt"][:, :, :],
                        in_=xd[ci, :, :].rearrange("p (t c) -> p t c", t=TCH),
                    )
                    nc.sync.dma_start(
                        out=state["at"][:, :, :],
                        in_=ad[ci, :, :].rearrange("p (t c) -> p t c", t=TCH),
                    )
                # x-side matmuls: no dependency on h(t-1); PE can run these early
                if "pe" not in ab:
                    for g in range(NG):
                        nc.tensor.matmul(
                            zps(g, 0), L_zx, X[g], start=True, stop=False
                        )
                    for g in range(NG):
                        nc.tensor.matmul(
                            zps(g, 1), L_rx, X[g], start=True, stop=False
                        )
                    for g in range(NG):
                        nc.tensor.matmul(
                            cps(g), L_hx, X[g], start=True, stop=False
                        )
                    # h-side accumulation. h(t-1) = m2 - m1p enters as two
                    # accumulating matmuls (m1p with negated weights), so the
                    # h materialization (DVE) is off the critical path.
                    if (t == 0 and state["m2"] is None) or mat_h:
                        for g in range(NG):
                            nc.tensor.matmul(
                                zps(g, 0), L_zh, H[g], start=False, stop=True
                            )
                            nc.tensor.matmul(
                                zps(g, 1), L_rh, H[g], start=False, stop=True
                            )
                    else:
                        pm1, pm2 = state["m1p"], state["m2"]
                        for g in range(NG):
                            nc.tensor.matmul(
                                zps(g, 0), L_zhn, pm1[g][:, :],
                                start=False, stop=False,
                            )
                            nc.tensor.matmul(
                                zps(g, 1), L_rhn, pm1[g][:, :],
                                start=False, stop=False,
                            )
                        for g in range(NG):
                            nc.tensor.matmul(
                                zps(g, 0), L_zh, pm2[g][:, :],
                                start=False, stop=True,
                            )
                            nc.tensor.matmul(
                                zps(g, 1), L_rh, pm2[g][:, :],
                                start=False, stop=True,
                            )

                zr, hz, ra, hc, m1p, m2 = ({} for _ in range(6))
                if act_ms:
                    zrm = wpool.tile([P, 2, 2, CG], mdt, tag="zr", name="zr")
                    if "act" not in ab:
                        nc.scalar.activation(
                            zrm[:, :, :, :], pzm[:, :, :, 0:CG], SIG
                        )
                    for g in range(NG):
                        zr[g] = zrm[:, g]
                elif "sigsplit" in ab:
                    for g in range(NG):
                        zr[g] = wpool.tile([P, 2, CG], mdt, tag="zr", name="zr")
                        nc.scalar.activation(zr[g][:, 0, :], pz[g][:, 0, 0:CG], SIG)
                    for g in range(NG):
                        nc.scalar.activation(zr[g][:, 1, :], pz[g][:, 1, 0:CG], SIG)
                else:
                    for g in range(NG):
                        zr[g] = wpool.tile([P, 2, CG], mdt, tag="zr", name="zr")
                        if "act" not in ab:
                            nc.scalar.activation(zr[g][:, :, :], pz[g][:, :, 0:CG], SIG)
                        if "act2" in ab:
                            nc.scalar.activation(zr[g][:, :, :], pz[g][:, :, 0:CG], SIG)
                for g in range(NG):
                    hz[g] = wpool.tile([P, CG], mdt, tag="hz", name="hz")
                    hz_eng = nc.gpsimd if "hzgp" in ab else nc.vector
                    if "dve" not in ab:
                        hz_eng.tensor_tensor(hz[g][:, :], H[g], zr[g][:, 0, :], MULT)
                    if "dve2" in ab:
                        hz_eng.tensor_tensor(hz[g][:, :], H[g], zr[g][:, 0, :], MULT)
                if "pe" not in ab:
                    for g in range(NG):
                        nc.tensor.matmul(
                            cps(g), L_ah, hz[g][:, :], start=False, stop=True
                        )
                for g in range(NG):
                    ra[g] = wpool.tile([P, CG], mdt, tag="ra", name="ra")
                    ra_eng = nc.gpsimd if "ragp" in ab else nc.vector
                    if "dve" not in ab:
                        ra_eng.tensor_tensor(ra[g][:, :], A[g], zr[g][:, 1, :], MULT)
                    if "dve2" in ab:
                        ra_eng.tensor_tensor(ra[g][:, :], A[g], zr[g][:, 1, :], MULT)
                for g in range(NG):
                    # m1p = (ra - 1) * h = -(1-ra)*h
                    m1p[g] = wpool.tile([P, CG], mdt, tag="m1p", name="m1p")
                    m1_eng = nc.gpsimd if "m1gp" in ab else nc.vector
                    if "dve" not in ab:
                        m1_eng.scalar_tensor_tensor(
                            m1p[g][:, :], ra[g][:, :], 1.0, H[g], SUB, MULT
                        )
                    if "dve2" in ab:
                        m1_eng.scalar_tensor_tensor(
                            m1p[g][:, :], ra[g][:, :], 1.0, H[g], SUB, MULT
                        )
                if act_mt:
                    hcm = wpool.tile([P, 2, CG], mdt, tag="hc", name="hc")
                    if "act" not in ab:
                        nc.scalar.activation(hcm[:, :, :], pxm[:, :, 0:CG], TANH)
                    for g in range(NG):
                        hc[g] = hcm[:, g]
                else:
                    for g in range(NG):
                        hc[g] = wpool.tile([P, CG], mdt, tag="hc", name="hc")
                        if "act" not in ab:
                            nc.scalar.activation(hc[g][:, :], px[g][:, 0:CG], TANH)
                        if "act2" in ab:
                            nc.scalar.activation(hc[g][:, :], px[g][:, 0:CG], TANH)
                for g in range(NG):
                    m2[g] = wpool.tile([P, CG], mdt, tag="m2", name="m2")
                    m2_eng = nc.gpsimd if "m2gp" in ab else nc.vector
                    if "dve" not in ab:
                        m2_eng.tensor_tensor(m2[g][:, :], ra[g][:, :], hc[g][:, :], MULT)
                    if "dve2" in ab:
                        m2_eng.tensor_tensor(m2[g][:, :], ra[g][:, :], hc[g][:, :], MULT)
                # materialized h' = m2 - m1p, off the matmul critical path.
                # DVE wins over gpsimd here: gpsimd TT latency (~1us) delays
                # next step's hz/m1p more than the extra DVE op costs.
                hp_eng = nc.gpsimd if "hpgp" in ab else nc.vector
                if "chain" not in ab and "gp" not in ab:
                    for g in range(NG):
                        hp_eng.tensor_tensor(HN[g], m2[g][:, :], m1p[g][:, :], SUB)
                if "gp2" in ab:
                    for g in range(NG):
                        nc.gpsimd.tensor_tensor(HN[g], m2[g][:, :], m1p[g][:, :], SUB)
                state["m1p"], state["m2"] = m1p, m2

        if loop and reps > 1:
            with tc.For_i(0, reps, 1):
                emit_body()
        else:
            for _ in range(reps):
                emit_body()

        nc.sync.dma_start(out=outd[:, :], in_=h_tiles[T % 2][:, :])

    nc.compile()
    return nc


def _get_program(reps=1, loop=False, ablate=()):
    key = ("nc", reps, loop, tuple(sorted(ablate)))
    if key not in _cache:
        _cache[key] = _build_program(reps, loop, ablate)
    return _cache[key]


def _effective_weights(inp):
    """Fold the per-gate double-dense into single 10x10 matrices (float64)."""
    g = {k: np.asarray(v, np.float64) for k, v in inp.items()}
    Wxz = g["Wi_z"] @ g["Ws_z"]; Az = g["Wh_z"] @ g["Ws_z"]
    bz = g["bi_z"] @ g["Ws_z"] + g["bs_z"]
    Wxr = g["Wi_r"] @ g["Ws_r"]; Ar = g["Wh_r"] @ g["Ws_r"]
    br = g["bi_r"] @ g["Ws_r"] + g["bs_r"]
    Wxh = g["Wi_h"] @ g["Wt_h"]; Ah = g["Wh_h"] @ g["Wt_h"]
    bh = g["bi_h"] @ g["Wt_h"] + g["bt_h"]
    return (Wxz, Az, bz), (Wxr, Ar, br), (Wxh, Ah, bh)


def _block_diag_lhsT(W, bias=None):
    """[P(+1), P] stationary: block-diagonal W per batch block, bias on ones row."""
    rows = P + 1 if bias is not None else P
    L = np.zeros((rows, P), np.float64)
    for q in range(NB):
        L[q * E:(q + 1) * E, q * E:(q + 1) * E] = W
        if bias is not None:
            L[P, q * E:(q + 1) * E] = bias
    return L


def _to_device_layout(arr, dt, ones_row=False):
    """[BPAD, T, E] -> [T//TCH, P(+1), TCH*NCOL]: chunk-major, contiguous
    per (chunk, partition) so each chunk DMA is one descriptor/partition."""
    out = arr.reshape(NB, NCOL, T, E).transpose(2, 0, 3, 1).reshape(T, P, NCOL)
    if ones_row:
        out = np.concatenate([out, np.ones((T, 1, NCOL), out.dtype)], axis=1)
    p = out.shape[1]
    # [T, p, NCOL] -> [NCH, TCH, p, NCOL] -> [NCH, p, TCH, NCOL] -> flat
    out = out.reshape(T // TCH, TCH, p, NCOL).transpose(0, 2, 1, 3).reshape(
        T // TCH, p, TCH * NCOL
    )
    return np.ascontiguousarray(out).astype(dt)


def _prep_in_maps(inputs):
    dt = _np_dtype()
    x = np.asarray(inputs["gru_hidden_state_inputs"], np.float32)
    a = np.asarray(inputs["attention_s"], np.float32)
    h0 = np.asarray(inputs["h0"], np.float32)

    (Wxz, Az, bz), (Wxr, Ar, br), (Wxh, Ah, bh) = _effective_weights(inputs)
    LWX = np.concatenate(
        [_block_diag_lhsT(Wxz, bz), _block_diag_lhsT(Wxr, br), _block_diag_lhsT(Wxh, bh)],
        axis=1,
    ).astype(dt)                                   # [121, 360]
    Lzh, Lrh, Lah = _block_diag_lhsT(Az), _block_diag_lhsT(Ar), _block_diag_lhsT(Ah)
    LWH = np.concatenate([Lzh, Lrh, Lah, -Lzh, -Lrh], axis=1).astype(dt)  # [120, 600]
    H0 = np.tile(h0.reshape(1, E, 1).astype(np.float64), (NB, 1, NCOL)).reshape(
        P, NCOL
    ).astype(dt)

    pad = BPAD - BC
    in_maps = []
    for c in range(NCORES):
        xc = np.concatenate([x[c * BC:(c + 1) * BC], np.zeros((pad, T, E), np.float32)])
        ac = np.concatenate([a[c * BC:(c + 1) * BC], np.zeros((pad, T, E), np.float32)])
        in_maps.append({
            "xd": _to_device_layout(xc, dt, ones_row=True),
            "ad": _to_device_layout(ac, dt),
            "h0d": H0,
            "lwx": LWX,
            "lwh": LWH,
        })
    return in_maps


def kernel(**inputs):
    in_maps = _prep_in_maps(inputs)
    nc = _get_program()
    put, run, _ = _get_runner(nc, 1)
    outs = run(put(in_maps))

    out = np.empty((B, E), np.float32)
    for c in range(NCORES):
        o = np.asarray(outs[c], np.float32)   # [P, NCOL]
        o = o.reshape(NB, E, NCOL).transpose(0, 2, 1).reshape(BPAD, E)
        out[c * BC:(c + 1) * BC] = o[:BC]
    return out


def _make_runner(nc):
    """Reusable jitted executor for `nc` (mirrors bass2jax.run_bass_via_pjrt
    multi-core path, but keeps the jitted fn for re-timing)."""
    import jax
    from jax.sharding import Mesh, PartitionSpec
    from jax.experimental.shard_map import shard_map
    from concourse import bass2jax, mybir

    bass2jax.install_neuronx_cc_hook()
    n_cores = NCORES

    in_names, out_names, out_avals = [], [], []
    for alloc in nc.m.functions[0].allocations:
        if not isinstance(alloc, mybir.MemoryLocationSet):
            continue
        name = alloc.memorylocations[0].name
        if alloc.kind == "ExternalInput":
            if nc.partition_id_tensor is not None and name == nc.partition_id_tensor.name:
                continue
            in_names.append(name)
        elif alloc.kind == "ExternalOutput":
            out_names.append(name)
            out_avals.append(
                jax.core.ShapedArray(tuple(alloc.tensor_shape), mybir.dt.np(alloc.dtype))
            )
    n_params = len(in_names)
    all_in_names = list(in_names) + list(out_names)
    if nc.partition_id_tensor is not None:
        all_in_names.append(nc.partition_id_tensor.name)

    def _body(*args):
        operands = list(args)
        if nc.partition_id_tensor is not None:
            operands.append(bass2jax.partition_id_tensor())
        outs = bass2jax._bass_exec_p.bind(
            *operands,
            out_avals=tuple(out_avals),
            in_names=tuple(all_in_names),
            out_names=tuple(out_names),
            lowering_input_output_aliases=(),
            sim_require_finite=True,
            sim_require_nnan=True,
            nc=nc,
        )
        return tuple(outs)

    devices = jax.devices()[:n_cores]
    mesh = Mesh(np.asarray(devices), ("core",))
    in_specs = (PartitionSpec("core"),) * (n_params + len(out_names))
    out_specs = (PartitionSpec("core"),) * len(out_names)
    sharded = jax.jit(
        shard_map(_body, mesh=mesh, in_specs=in_specs, out_specs=out_specs,
                  check_rep=False),
        keep_unused=True,
    )

    def put(in_maps):
        concat_in = [
            np.concatenate([np.asarray(in_maps[c][name]) for c in range(n_cores)], axis=0)
            for name in in_names
        ]
        concat_zeros = [
            np.zeros((n_cores * a.shape[0], *a.shape[1:]), a.dtype) for a in out_avals
        ]
        return [jax.device_put(a) for a in (*concat_in, *concat_zeros)]

    def run_device(dev_args):
        outs = sharded(*dev_args)
        return jax.block_until_ready(outs)

    def fetch(outs):
        o = np.asarray(outs[0]).reshape(n_cores, *out_avals[0].shape)
        return [o[c] for c in range(n_cores)]

    def run(dev_args):
        return fetch(run_device(dev_args))

    return put, run, run_device


def _get_runner(nc, key):
    k = ("runner", key)
    if k not in _cache:
        _cache[k] = _make_runner(nc)
    return _cache[k]



# revision 9
# speedup vs baseline: 1.3881x; 1.2355x over previous
"""AUGRU (attention-gated GRU) Trainium2 Bass kernel.

Math (per batch row b, per step t, E=10):
    r = sigmoid((x@Wi_r + bi_r + h@Wh_r) @ Ws_r + bs_r)
    z = sigmoid((x@Wi_z + bi_z + h@Wh_z) @ Ws_z + bs_z)
    hc = tanh((x@Wi_h + bi_h + (h*z)@Wh_h) @ Wt_h + bt_h)
    Ra = a * r
    h' = (1-Ra)*h + Ra*hc

The double-dense per gate folds algebraically:
    r = sigmoid(x@(Wi_r@Ws_r) + h@(Wh_r@Ws_r) + (bi_r@Ws_r + bs_r))
so only single effective 10x10 matrices reach the device.

Device layout (per core, batch shard BC=8192 padded to 8256):
  batch b -> block q = b // 688, column j = b % 688 (group g = j // 344)
  partition p = q*10 + e  (12 blocks x 10 feats = 120 partitions)
Matmuls are block-diagonal [120(+1 ones row), 120] stationaries on PE;
columns (batch) stream through. Host pre-transposes x/a to [T, 121, 688]
so all DMAs are contiguous. Two column groups (g=0,1) pipeline the serial
per-step dependency chain across engines.
"""

import os
from contextlib import ExitStack

import numpy as np

B, T, E = 65536, 50, 10
NCORES = 8
BC = B // NCORES          # 8192
NB = 12                   # batch blocks per core
P = NB * E                # 120 partitions
NCOL = 688                # columns per core (NB*NCOL = 8256 >= BC)
BPAD = NB * NCOL          # 8256
NG = 2                    # column pipeline groups
CG = NCOL // NG           # 344
TCH = 10                  # timesteps per X/A DMA chunk

_DT = os.environ.get("AUGRU_DT", "f16")   # f32 | f16 | bf16
_WBUFS = int(os.environ.get("AUGRU_WBUFS", "8"))

_cache = {}


def _np_dtype():
    if _DT == "f32":
        return np.float32
    if _DT == "f16":
        return np.float16
    import ml_dtypes
    return ml_dtypes.bfloat16


def _build_program(reps=1, loop=False, ablate=()):
    import concourse.bacc as bacc
    import concourse.tile as tile
    from concourse import mybir

    mdt = {
        "f32": mybir.dt.float32,
        "f16": mybir.dt.float16,
        "bf16": mybir.dt.bfloat16,
    }[_DT]
    f32 = mybir.dt.float32
    SIG = mybir.ActivationFunctionType.Sigmoid
    TANH = mybir.ActivationFunctionType.Tanh
    MULT = mybir.AluOpType.mult
    ADD = mybir.AluOpType.add
    SUB = mybir.AluOpType.subtract
    ab = set(ablate)

    nc = bacc.Bacc("TRN2", target_bir_lowering=False, debug=False)

    NCH = T // TCH
    xd = nc.dram_tensor("xd", [NCH, P + 1, TCH * NCOL], mdt, kind="ExternalInput")
    ad = nc.dram_tensor("ad", [NCH, P, TCH * NCOL], mdt, kind="ExternalInput")
    h0d = nc.dram_tensor("h0d", [P, NCOL], mdt, kind="ExternalInput")
    lwx = nc.dram_tensor("lwx", [P + 1, 3 * P], mdt, kind="ExternalInput")
    lwh = nc.dram_tensor("lwh", [P, 5 * P], mdt, kind="ExternalInput")
    outd = nc.dram_tensor("outd", [P, NCOL], mdt, kind="ExternalOutput")

    with tile.TileContext(nc) as tc, ExitStack() as ctx:
        cpool = ctx.enter_context(tc.tile_pool(name="consts", bufs=1))
        xpool = ctx.enter_context(tc.tile_pool(name="xbuf", bufs=2))
        apool = ctx.enter_context(tc.tile_pool(name="abuf", bufs=2))
        hpool = ctx.enter_context(tc.tile_pool(name="hbuf", bufs=1))
        wpool = ctx.enter_context(tc.tile_pool(name="work", bufs=_WBUFS))
        ppool = ctx.enter_context(tc.tile_pool(name="psum", bufs=2, space="PSUM"))

        # stationary weights, resident in SBUF
        lx = cpool.tile([P + 1, 3 * P], mdt, tag="lx", name="lx")
        lh = cpool.tile([P, 5 * P], mdt, tag="lh", name="lh")
        nc.sync.dma_start(out=lx[:, :], in_=lwx[:, :])
        nc.sync.dma_start(out=lh[:, :], in_=lwh[:, :])
        L_zx, L_rx, L_hx = (lx[:, i * P:(i + 1) * P] for i in range(3))
        L_zh, L_rh, L_ah, L_zhn, L_rhn = (lh[:, i * P:(i + 1) * P] for i in range(5))

        # recurrent state, parity double-buffered
        h_tiles = [
            hpool.tile([P, NCOL], mdt, tag=f"h{i}", name=f"h{i}") for i in range(2)
        ]
        nc.sync.dma_start(out=h_tiles[0][:, :], in_=h0d[:, :])
        if "chain" in ab:
            nc.sync.dma_start(out=h_tiles[1][:, :], in_=h0d[:, :])

        state = {"xt": None, "at": None, "m1p": None, "m2": None}
        mat_h = "matH" in ab         # z/r h-side from materialized h (2 MMs fewer)
        act_ms = "actmergeS" in ab   # one sigmoid instr across both groups
        act_mt = "actmergeT" in ab   # one tanh instr across both groups

        def emit_body():
            for t in range(T):
                if t % TCH == 0:
                    ci = t // TCH
                    state["xt"] = xpool.tile(
                        [P + 1, TCH, NCOL], mdt, tag="xt", name="xt"
                    )
                    nc.sync.dma_start(
                        out=state["xt"][:, :, :],
                        in_=xd[ci, :, :].rearrange("p (t c) -> p t c", t=TCH),
                    )
                    state["at"] = apool.tile([P, TCH, NCOL], mdt, tag="at", name="at")
                    nc.sync.dma_start(
                        out=state["at"][:, :, :],
                        in_=ad[ci, :, :].rearrange("p (t c) -> p t c", t=TCH),
                    )
                xt, at = state["xt"], state["at"]
                tt = t % TCH
                cur, nxt = h_tiles[t % 2], h_tiles[(t + 1) % 2]

                X = [xt[0:P + 1, tt, g * CG:(g + 1) * CG] for g in range(NG)]
                A = [at[:, tt, g * CG:(g + 1) * CG] for g in range(NG)]
                H = [cur[:, g * CG:(g + 1) * CG] for g in range(NG)]
                HN = [nxt[:, g * CG:(g + 1) * CG] for g in range(NG)]

                if act_ms:
                    pzm = ppool.tile([P, 2, 2, 512], f32, tag="pz", name="pz", bufs=1)
                    zps = lambda g, j, c=CG: pzm[:, g, j, 0:c]
                else:
                    # 3 bufs (6 of 8 psum banks): lets PE start step t+1's
                    # x-side matmuls while two earlier steps' pz are still
                    # being consumed
                    pzb = 2 if "pz2" in ab else 3
                    pz = [
                        ppool.tile([P, 2, 512], f32, tag="pz", name="pz", bufs=pzb)
                        for _ in range(NG)
                    ]
                    zps = lambda g, j, c=CG: pz[g][:, j, 0:c]
                if act_mt:
                    pxb = 1 if act_ms else 2
                    pxm = ppool.tile([P, 2, 512], f32, tag="px", name="px", bufs=pxb)
                    cps = lambda g, c=CG: pxm[:, g, 0:c]
                else:
                    px = [
                        ppool.tile([P, 512], f32, tag="px", name="px")
                        for _ in range(NG)
                    ]
                    cps = lambda g, c=CG: px[g][:, 0:c]

                if "pe2" in ab:
                    pdup = ppool.tile([P, 2, 512], f32, tag="pdup", name="pdup", bufs=1)
                    for g in range(NG):
                        nc.tensor.matmul(pdup[:, 0, 0:CG], L_zx, X[g], start=True, stop=True)
                        nc.tensor.matmul(pdup[:, 1, 0:CG], L_rx, X[g], start=True, stop=True)
                        nc.tensor.matmul(pdup[:, 0, 0:CG], L_hx, X[g], start=True, stop=True)
                        nc.tensor.matmul(pdup[:, 1, 0:CG], L_zh, H[g], start=True, stop=True)
                        nc.tensor.matmul(pdup[:, 0, 0:CG], L_rh, H[g], start=True, stop=True)
                        nc.tensor.matmul(pdup[:, 1, 0:CG], L_ah, H[g], start=True, stop=True)
                if "dma2" in ab and t % TCH == 0:
                    nc.sync.dma_start(
                        out=state["xt"][:, :, :],
                        in_=xd[ci, :, :].rearrange("p (t c) -> p t c", t=TCH),
                    )
                    nc.sync.dma_start(
                        out=state["at"][:, :, :],
                        in_=ad[ci, :, :].rearrange("p (t c) -> p t c", t=TCH),
                    )
                # x-side matmuls: no dependency on h(t-1); PE can run these early
                if "pe" not in ab:
                    for g in range(NG):
                        nc.tensor.matmul(
                            zps(g, 0), L_zx, X[g], start=True, stop=False
                        )
                    for g in range(NG):
                        nc.tensor.matmul(
                            zps(g, 1), L_rx, X[g], start=True, stop=False
                        )
                    for g in range(NG):
                        nc.tensor.matmul(
                            cps(g), L_hx, X[g], start=True, stop=False
                        )
                    # h-side accumulation. h(t-1) = m2 - m1p enters as two
                    # accumulating matmuls (m1p with negated weights), so the
                    # h materialization (DVE) is off the critical path.
                    if (t == 0 and state["m2"] is None) or mat_h:
                        for g in range(NG):
                            nc.tensor.matmul(
                                zps(g, 0), L_zh, H[g], start=False, stop=True
                            )
                            nc.tensor.matmul(
                                zps(g, 1), L_rh, H[g], start=False, stop=True
                            )
                    else:
                        pm1, pm2 = state["m1p"], state["m2"]
                        for g in range(NG):
                            nc.tensor.matmul(
                                zps(g, 0), L_zhn, pm1[g][:, :],
                                start=False, stop=False,
                            )
                            nc.tensor.matmul(
                                zps(g, 1), L_rhn, pm1[g][:, :],
                                start=False, stop=False,
                            )
                        for g in range(NG):
                            nc.tensor.matmul(
                                zps(g, 0), L_zh, pm2[g][:, :],
                                start=False, stop=True,
                            )
                            nc.tensor.matmul(
                                zps(g, 1), L_rh, pm2[g][:, :],
                                start=False, stop=True,
                            )

                zr, hz, ra, hc, m1p, m2 = ({} for _ in range(6))
                if act_ms:
                    zrm = wpool.tile([P, 2, 2, CG], mdt, tag="zr", name="zr")
                    if "act" not in ab:
                        nc.scalar.activation(
                            zrm[:, :, :, :], pzm[:, :, :, 0:CG], SIG
                        )
                    for g in range(NG):
                        zr[g] = zrm[:, g]
                elif "sigsplit" in ab:
                    for g in range(NG):
                        zr[g] = wpool.tile([P, 2, CG], mdt, tag="zr", name="zr")
                        nc.scalar.activation(zr[g][:, 0, :], pz[g][:, 0, 0:CG], SIG)
                    for g in range(NG):
                        nc.scalar.activation(zr[g][:, 1, :], pz[g][:, 1, 0:CG], SIG)
                else:
                    for g in range(NG):
                        zr[g] = wpool.tile([P, 2, CG], mdt, tag="zr", name="zr")
                        if "act" not in ab:
                            nc.scalar.activation(zr[g][:, :, :], pz[g][:, :, 0:CG], SIG)
                        if "act2" in ab:
                            nc.scalar.activation(zr[g][:, :, :], pz[g][:, :, 0:CG], SIG)
                for g in range(NG):
                    hz[g] = wpool.tile([P, CG], mdt, tag="hz", name="hz")
                    hz_eng = nc.gpsimd if "hzgp" in ab else nc.vector
                    if "dve" not in ab:
                        hz_eng.tensor_tensor(hz[g][:, :], H[g], zr[g][:, 0, :], MULT)
                    if "dve2" in ab:
                        hz_eng.tensor_tensor(hz[g][:, :], H[g], zr[g][:, 0, :], MULT)
                if "pe" not in ab:
                    for g in range(NG):
                        nc.tensor.matmul(
                            cps(g), L_ah, hz[g][:, :], start=False, stop=True
                        )
                for g in range(NG):
                    ra[g] = wpool.tile([P, CG], mdt, tag="ra", name="ra")
                    ra_eng = nc.gpsimd if "ragp" in ab else nc.vector
                    if "dve" not in ab:
                        ra_eng.tensor_tensor(ra[g][:, :], A[g], zr[g][:, 1, :], MULT)
                    if "dve2" in ab:
                        ra_eng.tensor_tensor(ra[g][:, :], A[g], zr[g][:, 1, :], MULT)
                for g in range(NG):
                    # m1p = (ra - 1) * h = -(1-ra)*h
                    m1p[g] = wpool.tile([P, CG], mdt, tag="m1p", name="m1p")
                    m1_eng = nc.gpsimd if "m1gp" in ab else nc.vector
                    if "dve" not in ab:
                        m1_eng.scalar_tensor_tensor(
                            m1p[g][:, :], ra[g][:, :], 1.0, H[g], SUB, MULT
                        )
                    if "dve2" in ab:
                        m1_eng.scalar_tensor_tensor(
                            m1p[g][:, :], ra[g][:, :], 1.0, H[g], SUB, MULT
                        )
                if act_mt:
                    hcm = wpool.tile([P, 2, CG], mdt, tag="hc", name="hc")
                    if "act" not in ab:
                        nc.scalar.activation(hcm[:, :, :], pxm[:, :, 0:CG], TANH)
                    for g in range(NG):
                        hc[g] = hcm[:, g]
                else:
                    for g in range(NG):
                        hc[g] = wpool.tile([P, CG], mdt, tag="hc", name="hc")
                        if "act" not in ab:
                            nc.scalar.activation(hc[g][:, :], px[g][:, 0:CG], TANH)
                        if "act2" in ab:
                            nc.scalar.activation(hc[g][:, :], px[g][:, 0:CG], TANH)
                for g in range(NG):
                    m2[g] = wpool.tile([P, CG], mdt, tag="m2", name="m2")
                    m2_eng = nc.gpsimd if "m2gp" in ab else nc.vector
                    if "dve" not in ab:
                        m2_eng.tensor_tensor(m2[g][:, :], ra[g][:, :], hc[g][:, :], MULT)
                    if "dve2" in ab:
                        m2_eng.tensor_tensor(m2[g][:, :], ra[g][:, :], hc[g][:, :], MULT)
                # materialized h' = m2 - m1p, off the matmul critical path.
                # DVE wins over gpsimd here: gpsimd TT latency (~1us) delays
                # next step's hz/m1p more than the extra DVE op costs.
                hp_eng = nc.gpsimd if "hpgp" in ab else nc.vector
                if "chain" not in ab and "gp" not in ab:
                    for g in range(NG):
                        hp_eng.tensor_tensor(HN[g], m2[g][:, :], m1p[g][:, :], SUB)
                if "gp2" in ab:
                    for g in range(NG):
                        nc.gpsimd.tensor_tensor(HN[g], m2[g][:, :], m1p[g][:, :], SUB)
                state["m1p"], state["m2"] = m1p, m2

        if loop and reps > 1:
            with tc.For_i(0, reps, 1):
                emit_body()
        else:
            for _ in range(reps):
                emit_body()

        nc.sync.dma_start(out=outd[:, :], in_=h_tiles[T % 2][:, :])

    nc.compile()
    return nc


def _get_program(reps=1, loop=False, ablate=()):
    key = ("nc", reps, loop, tuple(sorted(ablate)))
    if key not in _cache:
        _cache[key] = _build_program(reps, loop, ablate)
    return _cache[key]


def _effective_weights(inp):
    """Fold the per-gate double-dense into single 10x10 matrices (float64)."""
    g = {k: np.asarray(v, np.float64) for k, v in inp.items()}
    Wxz = g["Wi_z"] @ g["Ws_z"]; Az = g["Wh_z"] @ g["Ws_z"]
    bz = g["bi_z"] @ g["Ws_z"] + g["bs_z"]
    Wxr = g["Wi_r"] @ g["Ws_r"]; Ar = g["Wh_r"] @ g["Ws_r"]
    br = g["bi_r"] @ g["Ws_r"] + g["bs_r"]
    Wxh = g["Wi_h"] @ g["Wt_h"]; Ah = g["Wh_h"] @ g["Wt_h"]
    bh = g["bi_h"] @ g["Wt_h"] + g["bt_h"]
    return (Wxz, Az, bz), (Wxr, Ar, br), (Wxh, Ah, bh)


def _block_diag_lhsT(W, bias=None):
    """[P(+1), P] stationary: block-diagonal W per batch block, bias on ones row."""
    rows = P + 1 if bias is not None else P
    L = np.zeros((rows, P), np.float64)
    for q in range(NB):
        L[q * E:(q + 1) * E, q * E:(q + 1) * E] = W
        if bias is not None:
            L[P, q * E:(q + 1) * E] = bias
    return L


def _to_device_layout(arr, dt, ones_row=False):
    """[BPAD, T, E] -> [T//TCH, P(+1), TCH*NCOL]: chunk-major, contiguous
    per (chunk, partition) so each chunk DMA is one descriptor/partition."""
    out = arr.reshape(NB, NCOL, T, E).transpose(2, 0, 3, 1).reshape(T, P, NCOL)
    if ones_row:
        out = np.concatenate([out, np.ones((T, 1, NCOL), out.dtype)], axis=1)
    p = out.shape[1]
    # [T, p, NCOL] -> [NCH, TCH, p, NCOL] -> [NCH, p, TCH, NCOL] -> flat
    out = out.reshape(T // TCH, TCH, p, NCOL).transpose(0, 2, 1, 3).reshape(
        T // TCH, p, TCH * NCOL
    )
    return np.ascontiguousarray(out).astype(dt)


def _prep_in_maps(inputs):
    dt = _np_dtype()
    x = np.asarray(inputs["gru_hidden_state_inputs"], np.float32)
    a = np.asarray(inputs["attention_s"], np.float32)
    h0 = np.asarray(inputs["h0"], np.float32)

    (Wxz, Az, bz), (Wxr, Ar, br), (Wxh, Ah, bh) = _effective_weights(inputs)
    LWX = np.concatenate(
        [_block_diag_lhsT(Wxz, bz), _block_diag_lhsT(Wxr, br), _block_diag_lhsT(Wxh, bh)],
        axis=1,
    ).astype(dt)                                   # [121, 360]
    Lzh, Lrh, Lah = _block_diag_lhsT(Az), _block_diag_lhsT(Ar), _block_diag_lhsT(Ah)
    LWH = np.concatenate([Lzh, Lrh, Lah, -Lzh, -Lrh], axis=1).astype(dt)  # [120, 600]
    H0 = np.tile(h0.reshape(1, E, 1).astype(np.float64), (NB, 1, NCOL)).reshape(
        P, NCOL
    ).astype(dt)

    pad = BPAD - BC
    in_maps = []
    for c in range(NCORES):
        xc = np.concatenate([x[c * BC:(c + 1) * BC], np.zeros((pad, T, E), np.float32)])
        ac = np.concatenate([a[c * BC:(c + 1) * BC], np.zeros((pad, T, E), np.float32)])
        in_maps.append({
            "xd": _to_device_layout(xc, dt, ones_row=True),
            "ad": _to_device_layout(ac, dt),
            "h0d": H0,
            "lwx": LWX,
            "lwh": LWH,
        })
    return in_maps


def kernel(**inputs):
    in_maps = _prep_in_maps(inputs)
    nc = _get_program()
    put, run, _ = _get_runner(nc, 1)
    outs = run(put(in_maps))

    out = np.empty((B, E), np.float32)
    for c in range(NCORES):
        o = np.asarray(outs[c], np.float32)   # [P, NCOL]
        o = o.reshape(NB, E, NCOL).transpose(0, 2, 1).reshape(BPAD, E)
        out[c * BC:(c + 1) * BC] = o[:BC]
    return out


def _make_runner(nc):
    """Reusable jitted executor for `nc` (mirrors bass2jax.run_bass_via_pjrt
    multi-core path, but keeps the jitted fn for re-timing)."""
    import jax
    from jax.sharding import Mesh, PartitionSpec
    from jax.experimental.shard_map import shard_map
    from concourse import bass2jax, mybir

    bass2jax.install_neuronx_cc_hook()
    n_cores = NCORES

    in_names, out_names, out_avals = [], [], []
    for alloc in nc.m.functions[0].allocations:
        if not isinstance(alloc, mybir.MemoryLocationSet):
            continue
        name = alloc.memorylocations[0].name
        if alloc.kind == "ExternalInput":
            if nc.partition_id_tensor is not None and name == nc.partition_id_tensor.name:
                continue
            in_names.append(name)
        elif alloc.kind == "ExternalOutput":
            out_names.append(name)
            out_avals.append(
                jax.core.ShapedArray(tuple(alloc.tensor_shape), mybir.dt.np(alloc.dtype))
            )
    n_params = len(in_names)
    all_in_names = list(in_names) + list(out_names)
    if nc.partition_id_tensor is not None:
        all_in_names.append(nc.partition_id_tensor.name)

    def _body(*args):
        operands = list(args)
        if nc.partition_id_tensor is not None:
            operands.append(bass2jax.partition_id_tensor())
        outs = bass2jax._bass_exec_p.bind(
            *operands,
            out_avals=tuple(out_avals),
            in_names=tuple(all_in_names),
            out_names=tuple(out_names),
            lowering_input_output_aliases=(),
            sim_require_finite=True,
            sim_require_nnan=True,
            nc=nc,
        )
        return tuple(outs)

    devices = jax.devices()[:n_cores]
    mesh = Mesh(np.asarray(devices), ("core",))
    in_specs = (PartitionSpec("core"),) * (n_params + len(out_names))
    out_specs = (PartitionSpec("core"),) * len(out_names)
    sharded = jax.jit(
        shard_map(_body, mesh=mesh, in_specs=in_specs, out_specs=out_specs,
                  check_rep=False),
        keep_unused=True,
    )

    def put(in_maps):
        concat_in = [
            np.concatenate([np.asarray(in_maps[c][name]) for c in range(n_cores)], axis=0)
            for name in in_names
        ]
        concat_zeros = [
            np.zeros((n_cores * a.shape[0], *a.shape[1:]), a.dtype) for a in out_avals
        ]
        return [jax.device_put(a) for a in (*concat_in, *concat_zeros)]

    def run_device(dev_args):
        outs = sharded(*dev_args)
        return jax.block_until_ready(outs)

    def fetch(outs):
        o = np.asarray(outs[0]).reshape(n_cores, *out_avals[0].shape)
        return [o[c] for c in range(n_cores)]

    def run(dev_args):
        return fetch(run_device(dev_args))

    return put, run, run_device


def _get_runner(nc, key):
    k = ("runner", key)
    if k not in _cache:
        _cache[k] = _make_runner(nc)
    return _cache[k]



# revision 11
# speedup vs baseline: 1.4569x; 1.0496x over previous
"""AUGRU (attention-gated GRU) Trainium2 Bass kernel.

Math (per batch row b, per step t, E=10):
    r = sigmoid((x@Wi_r + bi_r + h@Wh_r) @ Ws_r + bs_r)
    z = sigmoid((x@Wi_z + bi_z + h@Wh_z) @ Ws_z + bs_z)
    hc = tanh((x@Wi_h + bi_h + (h*z)@Wh_h) @ Wt_h + bt_h)
    Ra = a * r
    h' = (1-Ra)*h + Ra*hc

The double-dense per gate folds algebraically:
    r = sigmoid(x@(Wi_r@Ws_r) + h@(Wh_r@Ws_r) + (bi_r@Ws_r + bs_r))
so only single effective 10x10 matrices reach the device.

Device layout (per core, batch shard BC=8192 padded to 8256):
  batch b -> block q = b // 688, column j = b % 688 (group g = j // 344)
  partition p = q*10 + e  (12 blocks x 10 feats = 120 partitions)
Matmuls are block-diagonal [120(+1 ones row), 120] stationaries on PE;
columns (batch) stream through. Host pre-transposes x/a to [T, 121, 688]
so all DMAs are contiguous. Two column groups (g=0,1) pipeline the serial
per-step dependency chain across engines.
"""

import os
from contextlib import ExitStack

import numpy as np

B, T, E = 65536, 50, 10
NCORES = 8
BC = B // NCORES          # 8192
NB = 12                   # batch blocks per core
P = NB * E                # 120 partitions
NCOL = 688                # columns per core (NB*NCOL = 8256 >= BC)
BPAD = NB * NCOL          # 8256
NG = 2                    # column pipeline groups
CG = NCOL // NG           # 344
TCH = 10                  # timesteps per X/A DMA chunk

_DT = os.environ.get("AUGRU_DT", "f16")   # f32 | f16 | bf16
_WBUFS = int(os.environ.get("AUGRU_WBUFS", "8"))

_cache = {}


def _np_dtype():
    if _DT == "f32":
        return np.float32
    if _DT == "f16":
        return np.float16
    import ml_dtypes
    return ml_dtypes.bfloat16


def _build_program(reps=1, loop=False, ablate=()):
    import concourse.bacc as bacc
    import concourse.tile as tile
    from concourse import mybir

    mdt = {
        "f32": mybir.dt.float32,
        "f16": mybir.dt.float16,
        "bf16": mybir.dt.bfloat16,
    }[_DT]
    f32 = mybir.dt.float32
    SIG = mybir.ActivationFunctionType.Sigmoid
    TANH = mybir.ActivationFunctionType.Tanh
    MULT = mybir.AluOpType.mult
    ADD = mybir.AluOpType.add
    SUB = mybir.AluOpType.subtract
    ab = set(ablate)

    nc = bacc.Bacc("TRN2", target_bir_lowering=False, debug=False)

    NCH = T // TCH
    xd = nc.dram_tensor("xd", [NCH, P + 1, TCH * NCOL], mdt, kind="ExternalInput")
    ad = nc.dram_tensor("ad", [NCH, P, TCH * NCOL], mdt, kind="ExternalInput")
    h0d = nc.dram_tensor("h0d", [P, NCOL], mdt, kind="ExternalInput")
    lwx = nc.dram_tensor("lwx", [P + 1, 3 * P], mdt, kind="ExternalInput")
    lwh = nc.dram_tensor("lwh", [P, 5 * P], mdt, kind="ExternalInput")
    outd = nc.dram_tensor("outd", [P, NCOL], mdt, kind="ExternalOutput")

    with tile.TileContext(nc) as tc, ExitStack() as ctx:
        cpool = ctx.enter_context(tc.tile_pool(name="consts", bufs=1))
        xpool = ctx.enter_context(tc.tile_pool(name="xbuf", bufs=2))
        apool = ctx.enter_context(tc.tile_pool(name="abuf", bufs=2))
        hpool = ctx.enter_context(tc.tile_pool(name="hbuf", bufs=1))
        wpool = ctx.enter_context(tc.tile_pool(name="work", bufs=_WBUFS))
        ppool = ctx.enter_context(tc.tile_pool(name="psum", bufs=2, space="PSUM"))

        # stationary weights, resident in SBUF
        lx = cpool.tile([P + 1, 3 * P], mdt, tag="lx", name="lx")
        lh = cpool.tile([P, 5 * P], mdt, tag="lh", name="lh")
        nc.sync.dma_start(out=lx[:, :], in_=lwx[:, :])
        nc.sync.dma_start(out=lh[:, :], in_=lwh[:, :])
        L_zx, L_rx, L_hx = (lx[:, i * P:(i + 1) * P] for i in range(3))
        L_zh, L_rh, L_ah, L_zhn, L_rhn = (lh[:, i * P:(i + 1) * P] for i in range(5))

        # recurrent state, parity double-buffered
        h_tiles = [
            hpool.tile([P, NCOL], mdt, tag=f"h{i}", name=f"h{i}") for i in range(2)
        ]
        nc.sync.dma_start(out=h_tiles[0][:, :], in_=h0d[:, :])
        if "chain" in ab:
            nc.sync.dma_start(out=h_tiles[1][:, :], in_=h0d[:, :])

        state = {"xt": None, "at": None, "m1p": None, "m2": None}
        mat_h = "matH" in ab         # z/r h-side from materialized h (2 MMs fewer)
        act_ms = "actmergeS" in ab   # one sigmoid instr across both groups
        act_mt = "actmergeT" in ab   # one tanh instr across both groups

        def emit_body():
            for t in range(T):
                if t % TCH == 0:
                    ci = t // TCH
                    state["xt"] = xpool.tile(
                        [P + 1, TCH, NCOL], mdt, tag="xt", name="xt"
                    )
                    nc.sync.dma_start(
                        out=state["xt"][:, :, :],
                        in_=xd[ci, :, :].rearrange("p (t c) -> p t c", t=TCH),
                    )
                    state["at"] = apool.tile([P, TCH, NCOL], mdt, tag="at", name="at")
                    nc.sync.dma_start(
                        out=state["at"][:, :, :],
                        in_=ad[ci, :, :].rearrange("p (t c) -> p t c", t=TCH),
                    )
                xt, at = state["xt"], state["at"]
                tt = t % TCH
                cur, nxt = h_tiles[t % 2], h_tiles[(t + 1) % 2]

                X = [xt[0:P + 1, tt, g * CG:(g + 1) * CG] for g in range(NG)]
                A = [at[:, tt, g * CG:(g + 1) * CG] for g in range(NG)]
                H = [cur[:, g * CG:(g + 1) * CG] for g in range(NG)]
                HN = [nxt[:, g * CG:(g + 1) * CG] for g in range(NG)]

                if act_ms:
                    pzm = ppool.tile([P, 2, 2, 512], f32, tag="pz", name="pz", bufs=1)
                    zps = lambda g, j, c=CG: pzm[:, g, j, 0:c]
                else:
                    # 3 bufs (6 of 8 psum banks): lets PE start step t+1's
                    # x-side matmuls while two earlier steps' pz are still
                    # being consumed
                    pzb = 2 if "pz2" in ab else 3
                    pz = [
                        ppool.tile([P, 2, 512], f32, tag="pz", name="pz", bufs=pzb)
                        for _ in range(NG)
                    ]
                    zps = lambda g, j, c=CG: pz[g][:, j, 0:c]
                if act_mt:
                    pxb = 1 if act_ms else 2
                    pxm = ppool.tile([P, 2, 512], f32, tag="px", name="px", bufs=pxb)
                    cps = lambda g, c=CG: pxm[:, g, 0:c]
                else:
                    px = [
                        ppool.tile([P, 512], f32, tag="px", name="px")
                        for _ in range(NG)
                    ]
                    cps = lambda g, c=CG: px[g][:, 0:c]

                if "pe2" in ab:
                    pdup = ppool.tile([P, 2, 512], f32, tag="pdup", name="pdup", bufs=1)
                    for g in range(NG):
                        nc.tensor.matmul(pdup[:, 0, 0:CG], L_zx, X[g], start=True, stop=True)
                        nc.tensor.matmul(pdup[:, 1, 0:CG], L_rx, X[g], start=True, stop=True)
                        nc.tensor.matmul(pdup[:, 0, 0:CG], L_hx, X[g], start=True, stop=True)
                        nc.tensor.matmul(pdup[:, 1, 0:CG], L_zh, H[g], start=True, stop=True)
                        nc.tensor.matmul(pdup[:, 0, 0:CG], L_rh, H[g], start=True, stop=True)
                        nc.tensor.matmul(pdup[:, 1, 0:CG], L_ah, H[g], start=True, stop=True)
                if "dma2" in ab and t % TCH == 0:
                    nc.sync.dma_start(
                        out=state["xt"][:, :, :],
                        in_=xd[ci, :, :].rearrange("p (t c) -> p t c", t=TCH),
                    )
                    nc.sync.dma_start(
                        out=state["at"][:, :, :],
                        in_=ad[ci, :, :].rearrange("p (t c) -> p t c", t=TCH),
                    )
                # x-side matmuls: no dependency on h(t-1); PE can run these early
                if "pe" not in ab:
                    for g in range(NG):
                        nc.tensor.matmul(
                            zps(g, 0), L_zx, X[g], start=True, stop=False
                        )
                    for g in range(NG):
                        nc.tensor.matmul(
                            zps(g, 1), L_rx, X[g], start=True, stop=False
                        )
                    for g in range(NG):
                        nc.tensor.matmul(
                            cps(g), L_hx, X[g], start=True, stop=False
                        )
                    # h-side accumulation. h(t-1) = m2 - m1p enters as two
                    # accumulating matmuls (m1p with negated weights), so the
                    # h materialization (DVE) is off the critical path.
                    if (t == 0 and state["m2"] is None) or mat_h:
                        for g in range(NG):
                            nc.tensor.matmul(
                                zps(g, 0), L_zh, H[g], start=False, stop=True
                            )
                            nc.tensor.matmul(
                                zps(g, 1), L_rh, H[g], start=False, stop=True
                            )
                    else:
                        pm1, pm2 = state["m1p"], state["m2"]
                        for g in range(NG):
                            nc.tensor.matmul(
                                zps(g, 0), L_zhn, pm1[g][:, :],
                                start=False, stop=False,
                            )
                            nc.tensor.matmul(
                                zps(g, 1), L_rhn, pm1[g][:, :],
                                start=False, stop=False,
                            )
                        for g in range(NG):
                            nc.tensor.matmul(
                                zps(g, 0), L_zh, pm2[g][:, :],
                                start=False, stop=True,
                            )
                            nc.tensor.matmul(
                                zps(g, 1), L_rh, pm2[g][:, :],
                                start=False, stop=True,
                            )

                zr, hz, ra, hc, m1p, m2 = ({} for _ in range(6))
                if act_ms:
                    zrm = wpool.tile([P, 2, 2, CG], mdt, tag="zr", name="zr")
                    if "act" not in ab:
                        nc.scalar.activation(
                            zrm[:, :, :, :], pzm[:, :, :, 0:CG], SIG
                        )
                    for g in range(NG):
                        zr[g] = zrm[:, g]
                elif "sigsplit" in ab:
                    for g in range(NG):
                        zr[g] = wpool.tile([P, 2, CG], mdt, tag="zr", name="zr")
                        nc.scalar.activation(zr[g][:, 0, :], pz[g][:, 0, 0:CG], SIG)
                    for g in range(NG):
                        nc.scalar.activation(zr[g][:, 1, :], pz[g][:, 1, 0:CG], SIG)
                else:
                    for g in range(NG):
                        zr[g] = wpool.tile([P, 2, CG], mdt, tag="zr", name="zr")
                        if "act" not in ab:
                            nc.scalar.activation(zr[g][:, :, :], pz[g][:, :, 0:CG], SIG)
                        if "act2" in ab:
                            nc.scalar.activation(zr[g][:, :, :], pz[g][:, :, 0:CG], SIG)
                for g in range(NG):
                    hz[g] = wpool.tile([P, CG], mdt, tag="hz", name="hz")
                    hz_eng = nc.gpsimd if "hzgp" in ab else nc.vector
                    if "dve" not in ab:
                        hz_eng.tensor_tensor(hz[g][:, :], H[g], zr[g][:, 0, :], MULT)
                    if "dve2" in ab:
                        hz_eng.tensor_tensor(hz[g][:, :], H[g], zr[g][:, 0, :], MULT)
                if "pe" not in ab:
                    for g in range(NG):
                        nc.tensor.matmul(
                            cps(g), L_ah, hz[g][:, :], start=False, stop=True
                        )
                for g in range(NG):
                    ra[g] = wpool.tile([P, CG], mdt, tag="ra", name="ra")
                    ra_eng = nc.gpsimd if "ragp" in ab else nc.vector
                    if "dve" not in ab:
                        ra_eng.tensor_tensor(ra[g][:, :], A[g], zr[g][:, 1, :], MULT)
                    if "dve2" in ab:
                        ra_eng.tensor_tensor(ra[g][:, :], A[g], zr[g][:, 1, :], MULT)
                rm1 = {}
                if "m1ts" in ab:
                    # (ra-1) via tensor_scalar (4x mode) then TT mult (2x mode)
                    # instead of one scalar_tensor_tensor (1x mode)
                    for g in range(NG):
                        rm1[g] = wpool.tile([P, CG], mdt, tag="rm1", name="rm1")
                        nc.vector.tensor_scalar_sub(rm1[g][:, :], ra[g][:, :], 1.0)
                for g in range(NG):
                    # m1p = (ra - 1) * h = -(1-ra)*h
                    m1p[g] = wpool.tile([P, CG], mdt, tag="m1p", name="m1p")
                    m1_eng = nc.gpsimd if "m1gp" in ab else nc.vector
                    if "m1ts" in ab:
                        m1_eng.tensor_tensor(m1p[g][:, :], rm1[g][:, :], H[g], MULT)
                    elif "dve" not in ab:
                        m1_eng.scalar_tensor_tensor(
                            m1p[g][:, :], ra[g][:, :], 1.0, H[g], SUB, MULT
                        )
                    if "dve2" in ab:
                        m1_eng.scalar_tensor_tensor(
                            m1p[g][:, :], ra[g][:, :], 1.0, H[g], SUB, MULT
                        )
                if act_mt:
                    hcm = wpool.tile([P, 2, CG], mdt, tag="hc", name="hc")
                    if "act" not in ab:
                        nc.scalar.activation(hcm[:, :, :], pxm[:, :, 0:CG], TANH)
                    for g in range(NG):
                        hc[g] = hcm[:, g]
                else:
                    for g in range(NG):
                        hc[g] = wpool.tile([P, CG], mdt, tag="hc", name="hc")
                        if "act" not in ab:
                            nc.scalar.activation(hc[g][:, :], px[g][:, 0:CG], TANH)
                        if "act2" in ab:
                            nc.scalar.activation(hc[g][:, :], px[g][:, 0:CG], TANH)
                for g in range(NG):
                    m2[g] = wpool.tile([P, CG], mdt, tag="m2", name="m2")
                    m2_eng = nc.gpsimd if "m2gp" in ab else nc.vector
                    if "dve" not in ab:
                        m2_eng.tensor_tensor(m2[g][:, :], ra[g][:, :], hc[g][:, :], MULT)
                    if "dve2" in ab:
                        m2_eng.tensor_tensor(m2[g][:, :], ra[g][:, :], hc[g][:, :], MULT)
                # materialized h' = m2 - m1p, off the matmul critical path.
                # DVE wins over gpsimd here: gpsimd TT latency (~1us) delays
                # next step's hz/m1p more than the extra DVE op costs.
                hp_eng = nc.gpsimd if "hpgp" in ab else nc.vector
                if "chain" not in ab and "gp" not in ab:
                    for g in range(NG):
                        eng = nc.gpsimd if ("hpmix" in ab and g == 0) else hp_eng
                        eng.tensor_tensor(HN[g], m2[g][:, :], m1p[g][:, :], SUB)
                if "gp2" in ab:
                    for g in range(NG):
                        nc.gpsimd.tensor_tensor(HN[g], m2[g][:, :], m1p[g][:, :], SUB)
                state["m1p"], state["m2"] = m1p, m2

        if loop and reps > 1:
            with tc.For_i(0, reps, 1):
                emit_body()
        else:
            for _ in range(reps):
                emit_body()

        nc.sync.dma_start(out=outd[:, :], in_=h_tiles[T % 2][:, :])

    nc.compile()
    return nc


def _get_program(reps=1, loop=False, ablate=()):
    key = ("nc", reps, loop, tuple(sorted(ablate)))
    if key not in _cache:
        _cache[key] = _build_program(reps, loop, ablate)
    return _cache[key]


def _effective_weights(inp):
    """Fold the per-gate double-dense into single 10x10 matrices (float64)."""
    g = {k: np.asarray(v, np.float64) for k, v in inp.items()}
    Wxz = g["Wi_z"] @ g["Ws_z"]; Az = g["Wh_z"] @ g["Ws_z"]
    bz = g["bi_z"] @ g["Ws_z"] + g["bs_z"]
    Wxr = g["Wi_r"] @ g["Ws_r"]; Ar = g["Wh_r"] @ g["Ws_r"]
    br = g["bi_r"] @ g["Ws_r"] + g["bs_r"]
    Wxh = g["Wi_h"] @ g["Wt_h"]; Ah = g["Wh_h"] @ g["Wt_h"]
    bh = g["bi_h"] @ g["Wt_h"] + g["bt_h"]
    return (Wxz, Az, bz), (Wxr, Ar, br), (Wxh, Ah, bh)


def _block_diag_lhsT(W, bias=None):
    """[P(+1), P] stationary: block-diagonal W per batch block, bias on ones row."""
    rows = P + 1 if bias is not None else P
    L = np.zeros((rows, P), np.float64)
    for q in range(NB):
        L[q * E:(q + 1) * E, q * E:(q + 1) * E] = W
        if bias is not None:
            L[P, q * E:(q + 1) * E] = bias
    return L


def _to_device_layout(arr, dt, ones_row=False):
    """[BPAD, T, E] -> [T//TCH, P(+1), TCH*NCOL]: chunk-major, contiguous
    per (chunk, partition) so each chunk DMA is one descriptor/partition."""
    out = arr.reshape(NB, NCOL, T, E).transpose(2, 0, 3, 1).reshape(T, P, NCOL)
    if ones_row:
        out = np.concatenate([out, np.ones((T, 1, NCOL), out.dtype)], axis=1)
    p = out.shape[1]
    # [T, p, NCOL] -> [NCH, TCH, p, NCOL] -> [NCH, p, TCH, NCOL] -> flat
    out = out.reshape(T // TCH, TCH, p, NCOL).transpose(0, 2, 1, 3).reshape(
        T // TCH, p, TCH * NCOL
    )
    return np.ascontiguousarray(out).astype(dt)


def _prep_in_maps(inputs):
    dt = _np_dtype()
    x = np.asarray(inputs["gru_hidden_state_inputs"], np.float32)
    a = np.asarray(inputs["attention_s"], np.float32)
    h0 = np.asarray(inputs["h0"], np.float32)

    (Wxz, Az, bz), (Wxr, Ar, br), (Wxh, Ah, bh) = _effective_weights(inputs)
    LWX = np.concatenate(
        [_block_diag_lhsT(Wxz, bz), _block_diag_lhsT(Wxr, br), _block_diag_lhsT(Wxh, bh)],
        axis=1,
    ).astype(dt)                                   # [121, 360]
    Lzh, Lrh, Lah = _block_diag_lhsT(Az), _block_diag_lhsT(Ar), _block_diag_lhsT(Ah)
    LWH = np.concatenate([Lzh, Lrh, Lah, -Lzh, -Lrh], axis=1).astype(dt)  # [120, 600]
    H0 = np.tile(h0.reshape(1, E, 1).astype(np.float64), (NB, 1, NCOL)).reshape(
        P, NCOL
    ).astype(dt)

    pad = BPAD - BC
    in_maps = []
    for c in range(NCORES):
        xc = np.concatenate([x[c * BC:(c + 1) * BC], np.zeros((pad, T, E), np.float32)])
        ac = np.concatenate([a[c * BC:(c + 1) * BC], np.zeros((pad, T, E), np.float32)])
        in_maps.append({
            "xd": _to_device_layout(xc, dt, ones_row=True),
            "ad": _to_device_layout(ac, dt),
            "h0d": H0,
            "lwx": LWX,
            "lwh": LWH,
        })
    return in_maps


def kernel(**inputs):
    in_maps = _prep_in_maps(inputs)
    nc = _get_program()
    put, run, _ = _get_runner(nc, 1)
    outs = run(put(in_maps))

    out = np.empty((B, E), np.float32)
    for c in range(NCORES):
        o = np.asarray(outs[c], np.float32)   # [P, NCOL]
        o = o.reshape(NB, E, NCOL).transpose(0, 2, 1).reshape(BPAD, E)
        out[c * BC:(c + 1) * BC] = o[:BC]
    return out


def _make_runner(nc):
    """Reusable jitted executor for `nc` (mirrors bass2jax.run_bass_via_pjrt
    multi-core path, but keeps the jitted fn for re-timing)."""
    import jax
    from jax.sharding import Mesh, PartitionSpec
    from jax.experimental.shard_map import shard_map
    from concourse import bass2jax, mybir

    bass2jax.install_neuronx_cc_hook()
    n_cores = NCORES

    in_names, out_names, out_avals = [], [], []
    for alloc in nc.m.functions[0].allocations:
        if not isinstance(alloc, mybir.MemoryLocationSet):
            continue
        name = alloc.memorylocations[0].name
        if alloc.kind == "ExternalInput":
            if nc.partition_id_tensor is not None and name == nc.partition_id_tensor.name:
                continue
            in_names.append(name)
        elif alloc.kind == "ExternalOutput":
            out_names.append(name)
            out_avals.append(
                jax.core.ShapedArray(tuple(alloc.tensor_shape), mybir.dt.np(alloc.dtype))
            )
    n_params = len(in_names)
    all_in_names = list(in_names) + list(out_names)
    if nc.partition_id_tensor is not None:
        all_in_names.append(nc.partition_id_tensor.name)

    def _body(*args):
        operands = list(args)
        if nc.partition_id_tensor is not None:
            operands.append(bass2jax.partition_id_tensor())
        outs = bass2jax._bass_exec_p.bind(
            *operands,
            out_avals=tuple(out_avals),
            in_names=tuple(all_in_names),
            out_names=tuple(out_names),
            lowering_input_output_aliases=(),
            sim_require_finite=True,
            sim_require_nnan=True,
            nc=nc,
        )
        return tuple(outs)

    devices = jax.devices()[:n_cores]
    mesh = Mesh(np.asarray(devices), ("core",))
    in_specs = (PartitionSpec("core"),) * (n_params + len(out_names))
    out_specs = (PartitionSpec("core"),) * len(out_names)
    sharded = jax.jit(
        shard_map(_body, mesh=mesh, in_specs=in_specs, out_specs=out_specs,
                  check_rep=False),
        keep_unused=True,
    )

    def put(in_maps):
        concat_in = [
            np.concatenate([np.asarray(in_maps[c][name]) for c in range(n_cores)], axis=0)
            for name in in_names
        ]
        concat_zeros = [
            np.zeros((n_cores * a.shape[0], *a.shape[1:]), a.dtype) for a in out_avals
        ]
        return [jax.device_put(a) for a in (*concat_in, *concat_zeros)]

    def run_device(dev_args):
        outs = sharded(*dev_args)
        return jax.block_until_ready(outs)

    def fetch(outs):
        o = np.asarray(outs[0]).reshape(n_cores, *out_avals[0].shape)
        return [o[c] for c in range(n_cores)]

    def run(dev_args):
        return fetch(run_device(dev_args))

    return put, run, run_device


def _get_runner(nc, key):
    k = ("runner", key)
    if k not in _cache:
        _cache[k] = _make_runner(nc)
    return _cache[k]

